# revision 18
# baseline (speedup 1.0000x reference)
"""3-layer GCN (B=4, N=4096, F=H=O=64) on 8 TRN2 NeuronCores.

Sharding: core c handles batch b=c//2, row-half r=c%2 (2048 rows of adj).
Host pre-transposes + bf16-casts each core's adj shard so the kernel can
keep it SBUF-resident (16MB) across all 3 GCN hops -> adj is read from
HBM exactly once. Between hops, the full node-feature matrix is
re-assembled with pair-wise AllGathers ([[0,1],[2,3],[4,5],[6,7]]),
chunked in two halves so the collective latency hides under the next
layer's matmuls on the already-gathered half.

Everything on the x-path is bf16 (fp32 matmuls cost 2x on the PE);
accumulation stays fp32 in PSUM and batchnorm runs fp32 from PSUM.
"""

import sys

sys.path.insert(0, "/opt/trn_rl_repo")

import numpy as np
import ml_dtypes

from concourse import bass, bacc, mybir, tile
from concourse.tile_rust import add_dep_helper
from concourse.bass_utils import run_bass_kernel_spmd


def _ensure_ntff_hook():
    """This image's ``antenv`` lacks ``axon_hooks``; shim it so
    ``run_bass_kernel_spmd(trace=True)`` can capture NTFF profiles (or at
    worst degrades to an untraced run instead of crashing on import)."""
    try:
        import antenv.axon_hooks  # noqa: F401
        return
    except ImportError:
        pass
    import types
    try:
        import antenv
    except ImportError:
        antenv = types.ModuleType("antenv")
        sys.modules["antenv"] = antenv
    mod = types.ModuleType("antenv.axon_hooks")
    holder = {"hook": None}
    mod.set_axon_ntff_profile_hook = lambda h: holder.__setitem__("hook", h)
    mod.get_axon_ntff_profile_hook = lambda: holder["hook"]
    sys.modules["antenv.axon_hooks"] = mod
    antenv.axon_hooks = mod
    try:
        from trn_agent_boot.trn_boot import _ntff_profile_via_ctypes
        hook = _ntff_profile_via_ctypes("/opt/axon/libaxon_pjrt.so")
        if hook is not None:
            mod.set_axon_ntff_profile_hook(hook)
    except Exception:
        pass


_ensure_ntff_hook()

F32 = mybir.dt.float32
BF16 = mybir.dt.bfloat16
BF16_NP = ml_dtypes.bfloat16

B, N = 4, 4096
NSH = N // 2          # rows per core (2048)
KC = N // 128         # contraction chunks of 128 (32)
NCH = NSH // 512      # output row chunks of 512 (4)
BN_EPS = 1e-3
N_CORES = 8
REPLICA_GROUPS = [[0, 1], [2, 3], [4, 5], [6, 7]]

# gather-chunk k-sets: chunk A covers xfull cols 0-1023 on both partition
# halves = k-chunks 0-7 (rank0 nodes) and 16-23 (rank1 nodes).
A_SET = list(range(0, 8)) + list(range(16, 24))
B_SET = list(range(8, 16)) + list(range(24, 32))


def _build():
    nc = bacc.Bacc("TRN2", target_bir_lowering=False, debug=False,
                   num_devices=N_CORES)

    adjt_d = nc.declare_dram_parameter("adjt", [N, NSH], BF16, isOutput=False)
    x0t_d = nc.declare_dram_parameter("x0t", [128, NSH], BF16, isOutput=False)
    # wblob: cols 0-191 = W1|W2|W3 (each [128,64], replicated on both
    # partition halves); cols 192-255 = Wl blocks (parts 0-63).
    wblob_d = nc.declare_dram_parameter("wblob", [128, 384], BF16,
                                        isOutput=False)
    # fblob: 10 cols of per-feature scalars: (bias,scale,shift) x3 + bl
    fblob_d = nc.declare_dram_parameter("fblob", [64, 16], F32,
                                        isOutput=False)
    out_d = nc.declare_dram_parameter("out", [64, NSH], F32, isOutput=True)

    dma_engs = None  # set inside context

    with tile.TileContext(nc) as tc:
        with (
            tc.tile_pool(name="const", bufs=1) as const,
            tc.tile_pool(name="adj", bufs=32) as adjp,
            tc.tile_pool(name="xfull", bufs=2) as xfullp,
            tc.tile_pool(name="xsh", bufs=1) as xshp,
            tc.tile_pool(name="pp", bufs=32) as ppool,
            tc.tile_pool(name="ost", bufs=2) as ostp,
            tc.tile_pool(name="ypsum", bufs=4, space="PSUM") as ypsump,
            tc.tile_pool(name="ppsum", bufs=4, space="PSUM") as ppsump,
            tc.tile_pool(name="dram", bufs=2, space="DRAM") as dram,
        ):
            # ---- inputs: x first (gates layer-1 P-step), blobs, then adjT
            # xfull is physically split into two [128,1024] tiles (one per
            # gather chunk) so layer L+1's A-group work only depends on
            # gather chunk A, not the whole gathered tensor.
            xfA = xfullp.tile([128, 1024], BF16, tag="xfa", name="xfa0")
            xfB = xfullp.tile([128, 1024], BF16, tag="xfb", name="xfb0")
            nc.scalar.dma_start(out=xfA[:], in_=x0t_d[:, 0:1024])
            nc.scalar.dma_start(out=xfB[:], in_=x0t_d[:, 1024:2048])

            wblob = const.tile([128, 384], BF16, tag="wblob", name="wblob")
            fblob = const.tile([64, 16], F32, tag="fblob", name="fblob")
            nc.scalar.dma_start(out=wblob[:], in_=wblob_d[:])
            nc.scalar.dma_start(out=fblob[:], in_=fblob_d[:])

            # prime the collective path early: ncfw's first-call entry
            # latency (~15us) is paid here, while the CC engine is idle
            # during the adjT load, instead of on the layer-1 gather.
            prime_i = dram.tile([64, 16], F32, tag="pri", name="prime_i")
            prime_o = dram.tile([2, 64, 16], F32, tag="pro", name="prime_o")
            prime_sb = ostp.tile([64, 16], F32, tag="prs", name="prime_sb")
            nc.vector.tensor_copy(prime_sb[:], fblob[:])
            nc.sync.dma_start(out=prime_i[:], in_=prime_sb[:])
            nc.gpsimd.collective_compute(
                "AllGather", mybir.AluOpType.bypass,
                replica_groups=REPLICA_GROUPS,
                ins=[prime_i.opt()], outs=[prime_o.opt()])

            adj_t = []
            issue = [nc.sync, nc.scalar]
            for k in range(KC):
                a = adjp.tile([128, NSH], BF16, tag="adj", name=f"adj{k}")
                issue[k % 2].dma_start(
                    out=a[:], in_=adjt_d[k * 128:(k + 1) * 128, :])
                adj_t.append(a)

            def w_sl(li, half):
                return wblob[64 * half:64 * half + 64, 64 * li:64 * li + 64]

            def wl_sl(i):
                return wblob[0:64, 192 + 64 * i:192 + 64 * (i + 1)]

            def f_sl(col):
                return fblob[:, col:col + 1]

            def p_step(li, k, xfa, xfb):
                half, off = (0, k) if k < KC // 2 else (1, k - KC // 2)
                xf, off = (xfa, off) if off < 8 else (xfb, off - 8)
                xsl = xf[64 * half:64 * half + 64, off * 128:(off + 1) * 128]
                pp = ppsump.tile([128, 64], F32, tag="pps", name=f"pp{li}_{k}")
                nc.tensor.matmul(pp[:], xsl, w_sl(li, half),
                                 start=True, stop=True)
                pk = ppool.tile([128, 64], BF16, tag="p", name=f"p{li}_{k}")
                nc.vector.tensor_copy(pk[:], pp[:])
                return pk

            def post(li, n, ys, xsh):
                sl = xsh[:, n * 512:(n + 1) * 512]
                nc.vector.tensor_scalar(
                    sl, ys[:], f_sl(3 * li), 0.0,
                    op0=mybir.AluOpType.add, op1=mybir.AluOpType.max)
                nc.vector.tensor_scalar(
                    sl, sl, f_sl(3 * li + 1), f_sl(3 * li + 2),
                    op0=mybir.AluOpType.mult, op1=mybir.AluOpType.add)

            def gather(li, half, xsh, xf_new):
                c0, c1 = half * 1024, (half + 1) * 1024
                bi = dram.tile([64, 1024], BF16, tag="bi", name=f"bi{li}_{half}")
                bo = dram.tile([2, 64, 1024], BF16, tag="bo",
                               name=f"bo{li}_{half}")
                nc.sync.dma_start(out=bi[:], in_=xsh[:, c0:c1])
                nc.gpsimd.collective_compute(
                    "AllGather", mybir.AluOpType.bypass,
                    replica_groups=REPLICA_GROUPS,
                    ins=[bi.opt()], outs=[bo.opt()])
                nc.scalar.dma_start(out=xf_new[0:64, :], in_=bo[0])
                nc.scalar.dma_start(out=xf_new[64:128, :], in_=bo[1])

            xsh_t = []

            # ================= layer 1: stream adjT, k-outer =================
            p_t = [p_step(0, k, xfA, xfB) for k in range(KC)]
            ys1 = [ypsump.tile([64, 512], F32, tag="y", name=f"y0_{n}")
                   for n in range(NCH)]
            # pass 0 (n=0,1) streams with the adjT DMA; pass 1 (n=2,3)
            # re-reads resident SBUF fast, overlapping gather-A latency.
            # Explicit order edges stop the scheduler from gap-filling
            # pass-1 matmuls into the DMA-paced pass-0 stream (that would
            # delay ys[0]/ys[1] completion and thus gather A).
            pass0_last, pass1_first = None, []
            for half in (0, 1):
                for k in range(KC):
                    for n in (2 * half, 2 * half + 1):
                        mm = nc.tensor.matmul(
                            ys1[n][:], p_t[k][:],
                            adj_t[k][:, n * 512:(n + 1) * 512],
                            start=(k == 0), stop=(k == KC - 1),
                            skip_group_check=True)
                        if half == 0:
                            pass0_last = mm
                        elif k == 0:
                            pass1_first.append(mm)
            for mm in pass1_first:
                add_dep_helper(mm.ins, pass0_last.ins, sync=False,
                               reason="L1 two-pass order")
            xsh1 = xshp.tile([64, NSH], BF16, tag="xsh0", name="xsh0")
            xsh_t.append(xsh1)
            xfA_nxt = xfullp.tile([128, 1024], BF16, tag="xfa", name="xfa1")
            xfB_nxt = xfullp.tile([128, 1024], BF16, tag="xfb", name="xfb1")
            for n in range(NCH):
                post(0, n, ys1[n], xsh1)
                if n == 1:
                    gather(0, 0, xsh1, xfA_nxt)
            gather(0, 1, xsh1, xfB_nxt)
            xfA, xfB = xfA_nxt, xfB_nxt

            # ============ layers 2,3: A/B groups, n-outer k-inner ============
            for li in (1, 2):
                pk = {}
                ys = [ypsump.tile([64, 512], F32, tag="y", name=f"y{li}_{n}")
                      for n in range(NCH)]
                for k in A_SET:
                    pk[k] = p_step(li, k, xfA, xfB)
                for n in range(NCH):
                    for k in A_SET:
                        nc.tensor.matmul(
                            ys[n][:], pk[k][:],
                            adj_t[k][:, n * 512:(n + 1) * 512],
                            start=(k == A_SET[0]), stop=False,
                            skip_group_check=True)
                for k in B_SET:
                    pk[k] = p_step(li, k, xfA, xfB)
                xsh = xshp.tile([64, NSH], BF16, tag=f"xsh{li}",
                                name=f"xsh{li}")
                xsh_t.append(xsh)
                if li < 2:
                    xfA_nxt = xfullp.tile([128, 1024], BF16, tag="xfa",
                                          name=f"xfa{li + 1}")
                    xfB_nxt = xfullp.tile([128, 1024], BF16, tag="xfb",
                                          name=f"xfb{li + 1}")
                for n in range(NCH):
                    for k in B_SET:
                        nc.tensor.matmul(
                            ys[n][:], pk[k][:],
                            adj_t[k][:, n * 512:(n + 1) * 512],
                            start=False, stop=(k == B_SET[-1]),
                            skip_group_check=True)
                    post(li, n, ys[n], xsh)
                    if li < 2 and n == 1:
                        gather(li, 0, xsh, xfA_nxt)
                if li < 2:
                    gather(li, 1, xsh, xfB_nxt)
                    xfA, xfB = xfA_nxt, xfB_nxt

            # ================= final linear =================
            for n in range(NCH):
                fs = ypsump.tile([64, 512], F32, tag="y", name=f"fs{n}")
                for i in range(3):
                    nc.tensor.matmul(
                        fs[:], wl_sl(i),
                        xsh_t[i][:, n * 512:(n + 1) * 512],
                        start=(i == 0), stop=(i == 2))
                ost = ostp.tile([64, 512], F32, tag="ost", name=f"ost{n}")
                nc.vector.tensor_scalar(
                    ost[:], fs[:], f_sl(9), 0.0,
                    op0=mybir.AluOpType.add, op1=mybir.AluOpType.max)
                nc.sync.dma_start(out=out_d[:, n * 512:(n + 1) * 512],
                                  in_=ost[:])

    nc.compile()
    return nc


_NC = None


def _get_nc():
    global _NC
    if _NC is None:
        _NC = _build()
    return _NC


def _make_in_maps(x, adj, W, b, g, be, m, v, Wl, bl):
    wblob = np.zeros((128, 384), np.float32)
    for i in range(3):
        wblob[0:64, 64 * i:64 * i + 64] = W[i]
        wblob[64:128, 64 * i:64 * i + 64] = W[i]
        wblob[0:64, 192 + 64 * i:192 + 64 * (i + 1)] = Wl[64 * i:64 * i + 64]
    wblob = wblob.astype(BF16_NP)
    fblob = np.zeros((64, 16), np.float32)
    for i in range(3):
        scale = g[i] / np.sqrt(v[i] + BN_EPS)
        fblob[:, 3 * i] = b[i]
        fblob[:, 3 * i + 1] = scale
        fblob[:, 3 * i + 2] = be[i] - m[i] * scale
    fblob[:, 9] = bl

    in_maps = []
    for c in range(N_CORES):
        bi, r = c // 2, c % 2
        rows = slice(r * NSH, (r + 1) * NSH)
        adjt = np.ascontiguousarray(adj[bi, rows, :].T).astype(BF16_NP)
        xb = x[bi]
        x0t = np.ascontiguousarray(
            np.concatenate([xb[:NSH].T, xb[NSH:].T], axis=0)).astype(BF16_NP)
        in_maps.append({
            "adjt": adjt, "x0t": x0t, "wblob": wblob, "fblob": fblob,
        })
    return in_maps


def kernel(x, adj, W1, b1, g1, be1, m1, v1, W2, b2, g2, be2, m2, v2,
           W3, b3, g3, be3, m3, v3, Wl, bl, _trace=False, _trace_kwargs=None):
    x = np.asarray(x, np.float32)
    adj = np.asarray(adj, np.float32)
    args = [np.asarray(a, np.float32) for a in
            (W1, b1, g1, be1, m1, v1, W2, b2, g2, be2, m2, v2,
             W3, b3, g3, be3, m3, v3, Wl, bl)]
    W = [args[0], args[6], args[12]]
    b = [args[1], args[7], args[13]]
    g = [args[2], args[8], args[14]]
    be = [args[3], args[9], args[15]]
    m = [args[4], args[10], args[16]]
    v = [args[5], args[11], args[17]]
    Wl_, bl_ = args[18], args[19]

    nc = _get_nc()
    in_maps = _make_in_maps(x, adj, W, b, g, be, m, v, Wl_, bl_)
    res = run_bass_kernel_spmd(
        nc, in_maps, list(range(N_CORES)),
        trace=_trace, **(_trace_kwargs or {}))

    out = np.zeros((B, N, 64), np.float32)
    for c in range(N_CORES):
        bi, r = c // 2, c % 2
        out[bi, r * NSH:(r + 1) * NSH, :] = res.results[c]["out"].T
    if _trace:
        kernel.last_result = res
    return out


# revision 19
# speedup vs baseline: 1.0410x; 1.0410x over previous
"""3-layer GCN (B=4, N=4096, F=H=O=64) on 8 TRN2 NeuronCores.

Sharding: core c handles batch b=c//2, row-half r=c%2 (2048 rows of adj).
Host pre-transposes + bf16-casts each core's adj shard so the kernel can
keep it SBUF-resident (16MB) across all 3 GCN hops -> adj is read from
HBM exactly once. Between hops, the full node-feature matrix is
re-assembled with pair-wise AllGathers ([[0,1],[2,3],[4,5],[6,7]]),
chunked in two halves so the collective latency hides under the next
layer's matmuls on the already-gathered half.

Everything on the x-path is bf16 (fp32 matmuls cost 2x on the PE);
accumulation stays fp32 in PSUM and batchnorm runs fp32 from PSUM.
"""

import sys

sys.path.insert(0, "/opt/trn_rl_repo")

import numpy as np
import ml_dtypes

from concourse import bass, bacc, mybir, tile
from concourse.tile_rust import add_dep_helper
from concourse.bass_utils import run_bass_kernel_spmd


def _ensure_ntff_hook():
    """This image's ``antenv`` lacks ``axon_hooks``; shim it so
    ``run_bass_kernel_spmd(trace=True)`` can capture NTFF profiles (or at
    worst degrades to an untraced run instead of crashing on import)."""
    try:
        import antenv.axon_hooks  # noqa: F401
        return
    except ImportError:
        pass
    import types
    try:
        import antenv
    except ImportError:
        antenv = types.ModuleType("antenv")
        sys.modules["antenv"] = antenv
    mod = types.ModuleType("antenv.axon_hooks")
    holder = {"hook": None}
    mod.set_axon_ntff_profile_hook = lambda h: holder.__setitem__("hook", h)
    mod.get_axon_ntff_profile_hook = lambda: holder["hook"]
    sys.modules["antenv.axon_hooks"] = mod
    antenv.axon_hooks = mod
    try:
        from trn_agent_boot.trn_boot import _ntff_profile_via_ctypes
        hook = _ntff_profile_via_ctypes("/opt/axon/libaxon_pjrt.so")
        if hook is not None:
            mod.set_axon_ntff_profile_hook(hook)
    except Exception:
        pass


_ensure_ntff_hook()

F32 = mybir.dt.float32
BF16 = mybir.dt.bfloat16
BF16_NP = ml_dtypes.bfloat16

B, N = 4, 4096
NSH = N // 2          # rows per core (2048)
KC = N // 128         # contraction chunks of 128 (32)
NCH = NSH // 512      # output row chunks of 512 (4)
BN_EPS = 1e-3
N_CORES = 8
REPLICA_GROUPS = [[0, 1], [2, 3], [4, 5], [6, 7]]

# gather-chunk k-sets: chunk A covers xfull cols 0-1023 on both partition
# halves = k-chunks 0-7 (rank0 nodes) and 16-23 (rank1 nodes).
A_SET = list(range(0, 8)) + list(range(16, 24))
B_SET = list(range(8, 16)) + list(range(24, 32))


def _build():
    nc = bacc.Bacc("TRN2", target_bir_lowering=False, debug=False,
                   num_devices=N_CORES)

    adjt_d = nc.declare_dram_parameter("adjt", [N, NSH], BF16, isOutput=False)
    x0t_d = nc.declare_dram_parameter("x0t", [128, NSH], BF16, isOutput=False)
    # wblob: cols 0-191 = W1|W2|W3 (each [128,64], replicated on both
    # partition halves); cols 192-255 = Wl blocks (parts 0-63).
    wblob_d = nc.declare_dram_parameter("wblob", [128, 384], BF16,
                                        isOutput=False)
    # fblob: 10 cols of per-feature scalars: (bias,scale,shift) x3 + bl
    fblob_d = nc.declare_dram_parameter("fblob", [64, 16], F32,
                                        isOutput=False)
    out_d = nc.declare_dram_parameter("out", [64, NSH], F32, isOutput=True)

    dma_engs = None  # set inside context

    with tile.TileContext(nc) as tc:
        with (
            tc.tile_pool(name="const", bufs=1) as const,
            tc.tile_pool(name="adj", bufs=32) as adjp,
            tc.tile_pool(name="xfull", bufs=2) as xfullp,
            tc.tile_pool(name="xsh", bufs=1) as xshp,
            tc.tile_pool(name="pp", bufs=32) as ppool,
            tc.tile_pool(name="ost", bufs=2) as ostp,
            tc.tile_pool(name="ypsum", bufs=4, space="PSUM") as ypsump,
            tc.tile_pool(name="ppsum", bufs=4, space="PSUM") as ppsump,
            tc.tile_pool(name="dram", bufs=2, space="DRAM") as dram,
        ):
            # ---- inputs: x first (gates layer-1 P-step), blobs, then adjT
            # xfull is physically split into two [128,1024] tiles (one per
            # gather chunk) so layer L+1's A-group work only depends on
            # gather chunk A, not the whole gathered tensor.
            xfA = xfullp.tile([128, 1024], BF16, tag="xfa", name="xfa0")
            xfB = xfullp.tile([128, 1024], BF16, tag="xfb", name="xfb0")
            nc.scalar.dma_start(out=xfA[:], in_=x0t_d[:, 0:1024])
            nc.scalar.dma_start(out=xfB[:], in_=x0t_d[:, 1024:2048])

            wblob = const.tile([128, 384], BF16, tag="wblob", name="wblob")
            fblob = const.tile([64, 16], F32, tag="fblob", name="fblob")
            nc.scalar.dma_start(out=wblob[:], in_=wblob_d[:])
            nc.scalar.dma_start(out=fblob[:], in_=fblob_d[:])

            adj_t = []
            issue = [nc.sync, nc.scalar]
            for k in range(KC):
                a = adjp.tile([128, NSH], BF16, tag="adj", name=f"adj{k}")
                issue[k % 2].dma_start(
                    out=a[:], in_=adjt_d[k * 128:(k + 1) * 128, :])
                adj_t.append(a)

            def w_sl(li, half):
                return wblob[64 * half:64 * half + 64, 64 * li:64 * li + 64]

            def wl_sl(i):
                return wblob[0:64, 192 + 64 * i:192 + 64 * (i + 1)]

            def f_sl(col):
                return fblob[:, col:col + 1]

            def p_step(li, k, xfa, xfb):
                half, off = (0, k) if k < KC // 2 else (1, k - KC // 2)
                xf, off = (xfa, off) if off < 8 else (xfb, off - 8)
                xsl = xf[64 * half:64 * half + 64, off * 128:(off + 1) * 128]
                pp = ppsump.tile([128, 64], F32, tag="pps", name=f"pp{li}_{k}")
                nc.tensor.matmul(pp[:], xsl, w_sl(li, half),
                                 start=True, stop=True)
                pk = ppool.tile([128, 64], BF16, tag="p", name=f"p{li}_{k}")
                nc.vector.tensor_copy(pk[:], pp[:])
                return pk

            def post(li, n, ys, xsh):
                sl = xsh[:, n * 512:(n + 1) * 512]
                nc.vector.tensor_scalar(
                    sl, ys[:], f_sl(3 * li), 0.0,
                    op0=mybir.AluOpType.add, op1=mybir.AluOpType.max)
                nc.vector.tensor_scalar(
                    sl, sl, f_sl(3 * li + 1), f_sl(3 * li + 2),
                    op0=mybir.AluOpType.mult, op1=mybir.AluOpType.add)

            def gather(li, half, xsh, xf_new):
                c0, c1 = half * 1024, (half + 1) * 1024
                bi = dram.tile([64, 1024], BF16, tag="bi", name=f"bi{li}_{half}")
                bo = dram.tile([2, 64, 1024], BF16, tag="bo",
                               name=f"bo{li}_{half}")
                nc.sync.dma_start(out=bi[:], in_=xsh[:, c0:c1])
                nc.gpsimd.collective_compute(
                    "AllGather", mybir.AluOpType.bypass,
                    replica_groups=REPLICA_GROUPS,
                    ins=[bi.opt()], outs=[bo.opt()])
                nc.scalar.dma_start(out=xf_new[0:64, :], in_=bo[0])
                nc.scalar.dma_start(out=xf_new[64:128, :], in_=bo[1])

            xsh_t = []

            # ================= layer 1: stream adjT, k-outer =================
            p_t = [p_step(0, k, xfA, xfB) for k in range(KC)]
            ys1 = [ypsump.tile([64, 512], F32, tag="y", name=f"y0_{n}")
                   for n in range(NCH)]
            # pass 0 (n=0,1) streams with the adjT DMA; pass 1 (n=2,3)
            # re-reads resident SBUF fast, overlapping gather-A latency.
            # Explicit order edges stop the scheduler from gap-filling
            # pass-1 matmuls into the DMA-paced pass-0 stream (that would
            # delay ys[0]/ys[1] completion and thus gather A).
            pass0_last, pass1_first = None, []
            for half in (0, 1):
                for k in range(KC):
                    for n in (2 * half, 2 * half + 1):
                        mm = nc.tensor.matmul(
                            ys1[n][:], p_t[k][:],
                            adj_t[k][:, n * 512:(n + 1) * 512],
                            start=(k == 0), stop=(k == KC - 1),
                            skip_group_check=True)
                        if half == 0:
                            pass0_last = mm
                        elif k == 0:
                            pass1_first.append(mm)

            xsh1 = xshp.tile([64, NSH], BF16, tag="xsh0", name="xsh0")
            xsh_t.append(xsh1)
            xfA_nxt = xfullp.tile([128, 1024], BF16, tag="xfa", name="xfa1")
            xfB_nxt = xfullp.tile([128, 1024], BF16, tag="xfb", name="xfb1")
            for n in range(NCH):
                post(0, n, ys1[n], xsh1)
                if n == 1:
                    gather(0, 0, xsh1, xfA_nxt)
            gather(0, 1, xsh1, xfB_nxt)
            xfA, xfB = xfA_nxt, xfB_nxt

            # ============ layers 2,3: A/B groups, n-outer k-inner ============
            for li in (1, 2):
                pk = {}
                ys = [ypsump.tile([64, 512], F32, tag="y", name=f"y{li}_{n}")
                      for n in range(NCH)]
                for k in A_SET:
                    pk[k] = p_step(li, k, xfA, xfB)
                for n in range(NCH):
                    for k in A_SET:
                        nc.tensor.matmul(
                            ys[n][:], pk[k][:],
                            adj_t[k][:, n * 512:(n + 1) * 512],
                            start=(k == A_SET[0]), stop=False,
                            skip_group_check=True)
                for k in B_SET:
                    pk[k] = p_step(li, k, xfA, xfB)
                xsh = xshp.tile([64, NSH], BF16, tag=f"xsh{li}",
                                name=f"xsh{li}")
                xsh_t.append(xsh)
                if li < 2:
                    xfA_nxt = xfullp.tile([128, 1024], BF16, tag="xfa",
                                          name=f"xfa{li + 1}")
                    xfB_nxt = xfullp.tile([128, 1024], BF16, tag="xfb",
                                          name=f"xfb{li + 1}")
                for n in range(NCH):
                    for k in B_SET:
                        nc.tensor.matmul(
                            ys[n][:], pk[k][:],
                            adj_t[k][:, n * 512:(n + 1) * 512],
                            start=False, stop=(k == B_SET[-1]),
                            skip_group_check=True)
                    post(li, n, ys[n], xsh)
                    if li < 2 and n == 1:
                        gather(li, 0, xsh, xfA_nxt)
                if li < 2:
                    gather(li, 1, xsh, xfB_nxt)
                    xfA, xfB = xfA_nxt, xfB_nxt

            # ================= final linear =================
            for n in range(NCH):
                fs = ypsump.tile([64, 512], F32, tag="y", name=f"fs{n}")
                for i in range(3):
                    nc.tensor.matmul(
                        fs[:], wl_sl(i),
                        xsh_t[i][:, n * 512:(n + 1) * 512],
                        start=(i == 0), stop=(i == 2))
                ost = ostp.tile([64, 512], F32, tag="ost", name=f"ost{n}")
                nc.vector.tensor_scalar(
                    ost[:], fs[:], f_sl(9), 0.0,
                    op0=mybir.AluOpType.add, op1=mybir.AluOpType.max)
                nc.sync.dma_start(out=out_d[:, n * 512:(n + 1) * 512],
                                  in_=ost[:])

    nc.compile()
    return nc


_NC = None


def _get_nc():
    global _NC
    if _NC is None:
        _NC = _build()
    return _NC


def _make_in_maps(x, adj, W, b, g, be, m, v, Wl, bl):
    wblob = np.zeros((128, 384), np.float32)
    for i in range(3):
        wblob[0:64, 64 * i:64 * i + 64] = W[i]
        wblob[64:128, 64 * i:64 * i + 64] = W[i]
        wblob[0:64, 192 + 64 * i:192 + 64 * (i + 1)] = Wl[64 * i:64 * i + 64]
    wblob = wblob.astype(BF16_NP)
    fblob = np.zeros((64, 16), np.float32)
    for i in range(3):
        scale = g[i] / np.sqrt(v[i] + BN_EPS)
        fblob[:, 3 * i] = b[i]
        fblob[:, 3 * i + 1] = scale
        fblob[:, 3 * i + 2] = be[i] - m[i] * scale
    fblob[:, 9] = bl

    in_maps = []
    for c in range(N_CORES):
        bi, r = c // 2, c % 2
        rows = slice(r * NSH, (r + 1) * NSH)
        adjt = np.ascontiguousarray(adj[bi, rows, :].T).astype(BF16_NP)
        xb = x[bi]
        x0t = np.ascontiguousarray(
            np.concatenate([xb[:NSH].T, xb[NSH:].T], axis=0)).astype(BF16_NP)
        in_maps.append({
            "adjt": adjt, "x0t": x0t, "wblob": wblob, "fblob": fblob,
        })
    return in_maps


def kernel(x, adj, W1, b1, g1, be1, m1, v1, W2, b2, g2, be2, m2, v2,
           W3, b3, g3, be3, m3, v3, Wl, bl, _trace=False, _trace_kwargs=None):
    x = np.asarray(x, np.float32)
    adj = np.asarray(adj, np.float32)
    args = [np.asarray(a, np.float32) for a in
            (W1, b1, g1, be1, m1, v1, W2, b2, g2, be2, m2, v2,
             W3, b3, g3, be3, m3, v3, Wl, bl)]
    W = [args[0], args[6], args[12]]
    b = [args[1], args[7], args[13]]
    g = [args[2], args[8], args[14]]
    be = [args[3], args[9], args[15]]
    m = [args[4], args[10], args[16]]
    v = [args[5], args[11], args[17]]
    Wl_, bl_ = args[18], args[19]

    nc = _get_nc()
    in_maps = _make_in_maps(x, adj, W, b, g, be, m, v, Wl_, bl_)
    res = run_bass_kernel_spmd(
        nc, in_maps, list(range(N_CORES)),
        trace=_trace, **(_trace_kwargs or {}))

    out = np.zeros((B, N, 64), np.float32)
    for c in range(N_CORES):
        bi, r = c // 2, c % 2
        out[bi, r * NSH:(r + 1) * NSH, :] = res.results[c]["out"].T
    if _trace:
        kernel.last_result = res
    return out


# revision 20
# speedup vs baseline: 1.1255x; 1.0811x over previous
"""3-layer GCN (B=4, N=4096, F=H=O=64) on 8 TRN2 NeuronCores.

Sharding: core c handles batch b=c//2, row-half r=c%2 (2048 rows of adj).
Host pre-transposes + bf16-casts each core's adj shard so the kernel can
keep it SBUF-resident (16MB) across all 3 GCN hops -> adj is read from
HBM exactly once. Between hops, the full node-feature matrix is
re-assembled with pair-wise AllGathers ([[0,1],[2,3],[4,5],[6,7]]),
chunked in two halves so the collective latency hides under the next
layer's matmuls on the already-gathered half.

Everything on the x-path is bf16 (fp32 matmuls cost 2x on the PE);
accumulation stays fp32 in PSUM and batchnorm runs fp32 from PSUM.
"""

import sys

sys.path.insert(0, "/opt/trn_rl_repo")

import numpy as np
import ml_dtypes

from concourse import bass, bacc, mybir, tile
from concourse.tile_rust import add_dep_helper
from concourse.bass_utils import run_bass_kernel_spmd


def _ensure_ntff_hook():
    """This image's ``antenv`` lacks ``axon_hooks``; shim it so
    ``run_bass_kernel_spmd(trace=True)`` can capture NTFF profiles (or at
    worst degrades to an untraced run instead of crashing on import)."""
    try:
        import antenv.axon_hooks  # noqa: F401
        return
    except ImportError:
        pass
    import types
    try:
        import antenv
    except ImportError:
        antenv = types.ModuleType("antenv")
        sys.modules["antenv"] = antenv
    mod = types.ModuleType("antenv.axon_hooks")
    holder = {"hook": None}
    mod.set_axon_ntff_profile_hook = lambda h: holder.__setitem__("hook", h)
    mod.get_axon_ntff_profile_hook = lambda: holder["hook"]
    sys.modules["antenv.axon_hooks"] = mod
    antenv.axon_hooks = mod
    try:
        from trn_agent_boot.trn_boot import _ntff_profile_via_ctypes
        hook = _ntff_profile_via_ctypes("/opt/axon/libaxon_pjrt.so")
        if hook is not None:
            mod.set_axon_ntff_profile_hook(hook)
    except Exception:
        pass


_ensure_ntff_hook()

F32 = mybir.dt.float32
BF16 = mybir.dt.bfloat16
BF16_NP = ml_dtypes.bfloat16

B, N = 4, 4096
NSH = N // 2          # rows per core (2048)
KC = N // 128         # contraction chunks of 128 (32)
NCH = NSH // 512      # output row chunks of 512 (4)
BN_EPS = 1e-3
N_CORES = 8
REPLICA_GROUPS = [[0, 1], [2, 3], [4, 5], [6, 7]]

# gather-chunk k-sets: chunk A covers xfull cols 0-1023 on both partition
# halves = k-chunks 0-7 (rank0 nodes) and 16-23 (rank1 nodes).
A_SET = list(range(0, 8)) + list(range(16, 24))
B_SET = list(range(8, 16)) + list(range(24, 32))


def _build():
    nc = bacc.Bacc("TRN2", target_bir_lowering=False, debug=False,
                   num_devices=N_CORES)

    adjt_d = nc.declare_dram_parameter("adjt", [N, NSH], BF16, isOutput=False)
    x0t_d = nc.declare_dram_parameter("x0t", [128, NSH], BF16, isOutput=False)
    # wblob: cols 0-191 = W1|W2|W3 (each [128,64], replicated on both
    # partition halves); cols 192-255 = Wl blocks (parts 0-63).
    wblob_d = nc.declare_dram_parameter("wblob", [128, 384], BF16,
                                        isOutput=False)
    # fblob: 10 cols of per-feature scalars: (bias,scale,shift) x3 + bl
    fblob_d = nc.declare_dram_parameter("fblob", [64, 16], F32,
                                        isOutput=False)
    out_d = nc.declare_dram_parameter("out", [64, NSH], F32, isOutput=True)

    dma_engs = None  # set inside context

    with tile.TileContext(nc) as tc:
        with (
            tc.tile_pool(name="const", bufs=1) as const,
            tc.tile_pool(name="adj", bufs=32) as adjp,
            tc.tile_pool(name="xfull", bufs=2) as xfullp,
            tc.tile_pool(name="xsh", bufs=1) as xshp,
            tc.tile_pool(name="pp", bufs=32) as ppool,
            tc.tile_pool(name="ost", bufs=2) as ostp,
            tc.tile_pool(name="ypsum", bufs=4, space="PSUM") as ypsump,
            tc.tile_pool(name="ppsum", bufs=4, space="PSUM") as ppsump,
            tc.tile_pool(name="dram", bufs=2, space="DRAM") as dram,
        ):
            # ---- inputs: x first (gates layer-1 P-step), blobs, then adjT
            # xfull is physically split into two [128,1024] tiles (one per
            # gather chunk) so layer L+1's A-group work only depends on
            # gather chunk A, not the whole gathered tensor.
            xfA = xfullp.tile([128, 1024], BF16, tag="xfa", name="xfa0")
            xfB = xfullp.tile([128, 1024], BF16, tag="xfb", name="xfb0")
            nc.scalar.dma_start(out=xfA[:], in_=x0t_d[:, 0:1024])
            nc.scalar.dma_start(out=xfB[:], in_=x0t_d[:, 1024:2048])

            wblob = const.tile([128, 384], BF16, tag="wblob", name="wblob")
            fblob = const.tile([64, 16], F32, tag="fblob", name="fblob")
            nc.scalar.dma_start(out=wblob[:], in_=wblob_d[:])
            nc.scalar.dma_start(out=fblob[:], in_=fblob_d[:])

            adj_t = []
            issue = [nc.sync, nc.scalar]
            for k in range(KC):
                a = adjp.tile([128, NSH], BF16, tag="adj", name=f"adj{k}")
                issue[k % 2].dma_start(
                    out=a[:], in_=adjt_d[k * 128:(k + 1) * 128, :])
                adj_t.append(a)

            def w_sl(li, half):
                return wblob[64 * half:64 * half + 64, 64 * li:64 * li + 64]

            def wl_sl(i):
                return wblob[0:64, 192 + 64 * i:192 + 64 * (i + 1)]

            def f_sl(col):
                return fblob[:, col:col + 1]

            def p_step(li, k, xfa, xfb):
                half, off = (0, k) if k < KC // 2 else (1, k - KC // 2)
                xf, off = (xfa, off) if off < 8 else (xfb, off - 8)
                xsl = xf[64 * half:64 * half + 64, off * 128:(off + 1) * 128]
                pp = ppsump.tile([128, 64], F32, tag="pps", name=f"pp{li}_{k}")
                nc.tensor.matmul(pp[:], xsl, w_sl(li, half),
                                 start=True, stop=True)
                pk = ppool.tile([128, 64], BF16, tag="p", name=f"p{li}_{k}")
                nc.vector.tensor_copy(pk[:], pp[:])
                return pk

            def post(li, n, ys, xsh):
                sl = xsh[:, n * 512:(n + 1) * 512]
                nc.vector.tensor_scalar(
                    sl, ys[:], f_sl(3 * li), 0.0,
                    op0=mybir.AluOpType.add, op1=mybir.AluOpType.max)
                nc.vector.tensor_scalar(
                    sl, sl, f_sl(3 * li + 1), f_sl(3 * li + 2),
                    op0=mybir.AluOpType.mult, op1=mybir.AluOpType.add)

            def gather(li, half, xsh, xf_new):
                c0, c1 = half * 1024, (half + 1) * 1024
                bi = dram.tile([64, 1024], BF16, tag="bi", name=f"bi{li}_{half}")
                bo = dram.tile([2, 64, 1024], BF16, tag="bo",
                               name=f"bo{li}_{half}")
                nc.sync.dma_start(out=bi[:], in_=xsh[:, c0:c1])
                nc.gpsimd.collective_compute(
                    "AllGather", mybir.AluOpType.bypass,
                    replica_groups=REPLICA_GROUPS,
                    ins=[bi.opt()], outs=[bo.opt()])
                nc.scalar.dma_start(out=xf_new[0:64, :], in_=bo[0])
                nc.scalar.dma_start(out=xf_new[64:128, :], in_=bo[1])

            xsh_t = []

            # ================= layer 1: stream adjT, k-outer =================
            p_t = [p_step(0, k, xfA, xfB) for k in range(KC)]
            ys1 = [ypsump.tile([64, 512], F32, tag="y", name=f"y0_{n}")
                   for n in range(NCH)]
            # pass 0 (n=0,1) streams with the adjT DMA; pass 1 (n=2,3)
            # re-reads resident SBUF fast, overlapping gather-A latency.
            # Explicit order edges stop the scheduler from gap-filling
            # pass-1 matmuls into the DMA-paced pass-0 stream (that would
            # delay ys[0]/ys[1] completion and thus gather A).
            pass0_last, pass1_first = None, []
            for half in (0, 1):
                for k in range(KC):
                    for n in (2 * half, 2 * half + 1):
                        mm = nc.tensor.matmul(
                            ys1[n][:], p_t[k][:],
                            adj_t[k][:, n * 512:(n + 1) * 512],
                            start=(k == 0), stop=(k == KC - 1),
                            skip_group_check=True)
                        if half == 0:
                            pass0_last = mm
                        elif k == 0:
                            pass1_first.append(mm)
            for mm in pass1_first:
                add_dep_helper(mm.ins, pass0_last.ins, sync=False,
                               reason="L1 two-pass order")
            xsh1 = xshp.tile([64, NSH], BF16, tag="xsh0", name="xsh0")
            xsh_t.append(xsh1)
            xfA_nxt = xfullp.tile([128, 1024], BF16, tag="xfa", name="xfa1")
            xfB_nxt = xfullp.tile([128, 1024], BF16, tag="xfb", name="xfb1")
            for n in range(NCH):
                post(0, n, ys1[n], xsh1)
                if n == 1:
                    gather(0, 0, xsh1, xfA_nxt)
            gather(0, 1, xsh1, xfB_nxt)
            xfA, xfB = xfA_nxt, xfB_nxt

            # ============ layers 2,3: A/B groups, n-outer k-inner ============
            for li in (1, 2):
                pk = {}
                ys = [ypsump.tile([64, 512], F32, tag="y", name=f"y{li}_{n}")
                      for n in range(NCH)]
                for k in A_SET:
                    pk[k] = p_step(li, k, xfA, xfB)
                for n in range(NCH):
                    for k in A_SET:
                        nc.tensor.matmul(
                            ys[n][:], pk[k][:],
                            adj_t[k][:, n * 512:(n + 1) * 512],
                            start=(k == A_SET[0]), stop=False,
                            skip_group_check=True)
                for k in B_SET:
                    pk[k] = p_step(li, k, xfA, xfB)
                xsh = xshp.tile([64, NSH], BF16, tag=f"xsh{li}",
                                name=f"xsh{li}")
                xsh_t.append(xsh)
                if li < 2:
                    xfA_nxt = xfullp.tile([128, 1024], BF16, tag="xfa",
                                          name=f"xfa{li + 1}")
                    xfB_nxt = xfullp.tile([128, 1024], BF16, tag="xfb",
                                          name=f"xfb{li + 1}")
                for n in range(NCH):
                    for k in B_SET:
                        nc.tensor.matmul(
                            ys[n][:], pk[k][:],
                            adj_t[k][:, n * 512:(n + 1) * 512],
                            start=False, stop=(k == B_SET[-1]),
                            skip_group_check=True)
                    post(li, n, ys[n], xsh)
                    if li < 2 and n == 1:
                        gather(li, 0, xsh, xfA_nxt)
                if li < 2:
                    gather(li, 1, xsh, xfB_nxt)
                    xfA, xfB = xfA_nxt, xfB_nxt

            # ================= final linear =================
            for n in range(NCH):
                fs = ypsump.tile([64, 512], F32, tag="y", name=f"fs{n}")
                for i in range(3):
                    nc.tensor.matmul(
                        fs[:], wl_sl(i),
                        xsh_t[i][:, n * 512:(n + 1) * 512],
                        start=(i == 0), stop=(i == 2))
                ost = ostp.tile([64, 512], F32, tag="ost", name=f"ost{n}")
                nc.vector.tensor_scalar(
                    ost[:], fs[:], f_sl(9), 0.0,
                    op0=mybir.AluOpType.add, op1=mybir.AluOpType.max)
                nc.sync.dma_start(out=out_d[:, n * 512:(n + 1) * 512],
                                  in_=ost[:])

    nc.compile()
    return nc


_NC = None


def _get_nc():
    global _NC
    if _NC is None:
        _NC = _build()
    return _NC


def _make_in_maps(x, adj, W, b, g, be, m, v, Wl, bl):
    wblob = np.zeros((128, 384), np.float32)
    for i in range(3):
        wblob[0:64, 64 * i:64 * i + 64] = W[i]
        wblob[64:128, 64 * i:64 * i + 64] = W[i]
        wblob[0:64, 192 + 64 * i:192 + 64 * (i + 1)] = Wl[64 * i:64 * i + 64]
    wblob = wblob.astype(BF16_NP)
    fblob = np.zeros((64, 16), np.float32)
    for i in range(3):
        scale = g[i] / np.sqrt(v[i] + BN_EPS)
        fblob[:, 3 * i] = b[i]
        fblob[:, 3 * i + 1] = scale
        fblob[:, 3 * i + 2] = be[i] - m[i] * scale
    fblob[:, 9] = bl

    in_maps = []
    for c in range(N_CORES):
        bi, r = c // 2, c % 2
        rows = slice(r * NSH, (r + 1) * NSH)
        adjt = np.ascontiguousarray(adj[bi, rows, :].T).astype(BF16_NP)
        xb = x[bi]
        x0t = np.ascontiguousarray(
            np.concatenate([xb[:NSH].T, xb[NSH:].T], axis=0)).astype(BF16_NP)
        in_maps.append({
            "adjt": adjt, "x0t": x0t, "wblob": wblob, "fblob": fblob,
        })
    return in_maps


def kernel(x, adj, W1, b1, g1, be1, m1, v1, W2, b2, g2, be2, m2, v2,
           W3, b3, g3, be3, m3, v3, Wl, bl, _trace=False, _trace_kwargs=None):
    x = np.asarray(x, np.float32)
    adj = np.asarray(adj, np.float32)
    args = [np.asarray(a, np.float32) for a in
            (W1, b1, g1, be1, m1, v1, W2, b2, g2, be2, m2, v2,
             W3, b3, g3, be3, m3, v3, Wl, bl)]
    W = [args[0], args[6], args[12]]
    b = [args[1], args[7], args[13]]
    g = [args[2], args[8], args[14]]
    be = [args[3], args[9], args[15]]
    m = [args[4], args[10], args[16]]
    v = [args[5], args[11], args[17]]
    Wl_, bl_ = args[18], args[19]

    nc = _get_nc()
    in_maps = _make_in_maps(x, adj, W, b, g, be, m, v, Wl_, bl_)
    res = run_bass_kernel_spmd(
        nc, in_maps, list(range(N_CORES)),
        trace=_trace, **(_trace_kwargs or {}))

    out = np.zeros((B, N, 64), np.float32)
    for c in range(N_CORES):
        bi, r = c // 2, c % 2
        out[bi, r * NSH:(r + 1) * NSH, :] = res.results[c]["out"].T
    if _trace:
        kernel.last_result = res
    return out
